# revision 42
# baseline (speedup 1.0000x reference)
"""Cross-attention Trainium2 kernel, 8 NeuronCores, head-parallel sharding.

Reference computation (fp32):
    q = x @ Wq; k = cond @ Wk; v = cond @ Wv        (per-head dh=40, 8 heads)
    attn = softmax(q k^T / sqrt(dh)); out = (attn v) @ Wo + bo

Sharding: 16 (batch, head) pairs across 8 cores -> core c handles batch c//4,
heads 2*(c%4), 2*(c%4)+1.  Each core computes a partial [S, D_MODEL] output
(its two heads' contribution through Wo, as bf16); the host sums the 4
partials per batch in fp32 and adds the bias.

Datapath (bf16 activations, fp32 PSUM accumulation):
  - all weights/identities packed into one [128, 2224] bf16 "wall" tensor so
    a single DMA loads them (dma_start holds its queue's sequencer until the
    transfer completes, so many small weight DMAs would serialize ~25us).
  - scores S^T[keys, q] = kT_chunk^T @ qT (128-key stationary, 512-q moving)
  - exp alternates between the Activation engine (exact exp, bf16 out) and
    the DVE (Schraudolph bit-trick: round(A*s+B) as int16, bits reinterpreted
    as bf16 ~ exp(s*scale), ~1.8% rms) so both engines share the 33.5M-elem
    softmax exponential, which is otherwise the serial bottleneck.
  - AV uses P^T chunks as the *stationary* operand ([128k x 128q]) and
    V-natural chunks (+ ones column for the denominator) as the 41-wide
    moving operand: 41 cycles/matmul instead of 512 (stationary loads are
    free), cutting AV PE time 3x vs the V-stationary orientation.
  - K and V are projected in natural [seq, dh] orientation together (one
    [128,160] matmul stream), V slices feed AV directly, K chunks are
    PE-transposed into kT.
  - PSUM allows one accumulation group per 2KB bank (start=True zeroes the
    whole bank): each attention pair owns a bank whose 4 q-subchunk AV
    accumulations form a single group; the same bank's tail region is reused
    as the transpose scratch only after every normalize has read the values.
"""

import sys

for _p in ("/opt/trn_rl_repo", "/root/.axon_site/_ro/trn_rl_repo"):
    if _p not in sys.path:
        sys.path.append(_p)

import math
import numpy as np

B, S, SK = 2, 4096, 4096
DM, DC, H, DH = 320, 768, 8, 40
NCORES = 8
QB = 512               # query block (psum bank width fp32)
NQB = S // QB          # 8
KC = 128               # key chunk (psum partitions)
NKC = SK // KC         # 32
GRP = 2                # key chunks per exp group
NG = NKC // GRP        # groups per (qb, h)
SCALE = DH ** -0.5
VW = 2 * DH + 2        # vaug row: [ones | V0(40) | V1(40) | ones]
A_EXP = (2.0 ** 7) / math.log(2.0) * SCALE
B_EXP = 16248.6

# wall (packed weights) column offsets, all bf16
W_WKV = 0                       # 6 chunks x [128, 160]
W_WQ = W_WKV + 6 * 160          # 2 heads x 3 chunks x [128, 40]
W_WO = W_WQ + 6 * 40            # 2 heads x [40, 320]
W_EYE = W_WO + 2 * 320          # [128, 128] bf16
W_EYER = W_EYE + 128            # [128, 128] f32r as 256 bf16-bit cols
W_COLS = W_EYER + 256           # 2224

# exp engine per group in the flat (qb,h,g) stream: 0=ACT exact, 1=DVE approx
EXP_PAT = (0, 1)

_CACHE = {}


def _build_nc():
    import concourse.mybir as mybir
    import concourse.tile as tile
    from concourse import bacc
    from concourse.alu_op_type import AluOpType

    F32 = mybir.dt.float32
    F32R = mybir.dt.float32r
    BF16 = mybir.dt.bfloat16
    I16 = mybir.dt.int16
    EXPF = mybir.ActivationFunctionType.Exp

    nc = bacc.Bacc(None, target_bir_lowering=False)

    wall_d = nc.dram_tensor("wall", [128, W_COLS], BF16, kind="ExternalInput")
    xT = nc.dram_tensor("xT", [128, 3, S], BF16, kind="ExternalInput")
    condT = nc.dram_tensor("condT", [128, 6, SK], BF16, kind="ExternalInput")
    out_d = nc.dram_tensor("out", [S, DM], BF16, kind="ExternalOutput")

    with tile.TileContext(nc) as tc:
      with tc.tile_pool(name="persist", bufs=1) as pp:
        wall = pp.tile([128, W_COLS], BF16, tag="wall", name="wall")
        nc.gpsimd.dma_start(wall[:], wall_d[:])
        wkv_t = [wall[:, W_WKV + i * 160:W_WKV + (i + 1) * 160]
                 for i in range(6)]
        wq_t = [[wall[:, W_WQ + (3 * h + i) * 40:W_WQ + (3 * h + i + 1) * 40]
                 for i in range(3)] for h in range(2)]
        wo_t = [wall[0:DH, W_WO + h * 320:W_WO + (h + 1) * 320]
                for h in range(2)]
        eye_t = wall[:, W_EYE:W_EYE + 128]

        # ---- persistent activations ----
        kT = [pp.tile([DH, SK], BF16, tag=f"kT{h}", name=f"kT{h}")
              for h in range(2)]
        qT = [pp.tile([DH, S], BF16, tag=f"qT{h}", name=f"qT{h}")
              for h in range(2)]
        outT = [pp.tile([DH, S], BF16, tag=f"oT{h}", name=f"oT{h}")
                for h in range(2)]
        vaug = pp.tile([128, NKC, VW], BF16, tag="vaug", name="vaug")
        nc.vector.memset(vaug[:, :, 0:1], 1.0)
        nc.vector.memset(vaug[:, :, VW - 1:VW], 1.0)

        # inputs: one big x DMA, cond in 4 sk-quarter DMAs (so K/V work can
        # start before the full cond transfer lands)
        xc = pp.tile([128, 3, S], BF16, tag="xc", name="xc")
        cc = pp.tile([128, 6, SK], BF16, tag="cc", name="cc")
        nc.sync.dma_start(xc[:, :, 0:S // 2], xT[:, :, 0:S // 2])
        nc.sync.dma_start(xc[:, :, S // 2:S], xT[:, :, S // 2:S])
        SKQ = SK // 4
        for q4 in range(4):
            nc.sync.dma_start(cc[:, :, q4 * SKQ:(q4 + 1) * SKQ],
                              condT[:, :, q4 * SKQ:(q4 + 1) * SKQ])

        # ---- phase Q: qT = (x @ Wq)^T per head (col-blocks first so the
        # first x half-DMA unblocks half the blocks) ----
        with tc.tile_pool(name="qps", bufs=3, space="PSUM") as qpp:
            for nb in range(NQB):
                for h in range(2):
                    ps = qpp.tile([DH, QB], F32, tag="qps", name="qps")
                    for i in range(3):
                        nc.tensor.matmul(ps[:], wq_t[h][i],
                                         xc[:, i, nb * QB:(nb + 1) * QB],
                                         start=(i == 0), stop=(i == 2))
                    nc.scalar.copy(qT[h][:, nb * QB:(nb + 1) * QB], ps[:])

        # ---- phase KV: natural K/V (single-pass accumulation) ----
        with (
            tc.tile_pool(name="kvsb", bufs=3) as kvp,
            tc.tile_pool(name="kvps", bufs=3, space="PSUM") as kbp,
            tc.tile_pool(name="tps", bufs=2, space="PSUM") as tpp,
        ):
            for k in range(NKC):
                ps = kbp.tile([128, 4 * DH], F32, tag="kvb", name="kvb")
                for i in range(6):
                    nc.tensor.matmul(ps[:], cc[:, i, k * KC:(k + 1) * KC],
                                     wkv_t[i], start=(i == 0), stop=(i == 5))
                kv = kvp.tile([128, 2 * DH], BF16, tag="kv", name="kv")
                nc.scalar.copy(kv[:], ps[:, 0:2 * DH])
                # V both heads -> vaug cols 1..81 straight from psum
                with nc.allow_low_precision(reason="bf16 v"):
                    nc.vector.tensor_copy(vaug[:, k, 1:1 + 2 * DH],
                                          ps[:, 2 * DH:4 * DH])
                for h in range(2):
                    t = tpp.tile([DH, KC], BF16, tag="tp", name="tp")
                    nc.tensor.transpose(t[:], kv[:, h * DH:(h + 1) * DH],
                                        eye_t)
                    if h == 0:
                        nc.scalar.copy(kT[h][:, k * KC:(k + 1) * KC], t[:])
                    else:
                        nc.vector.tensor_copy(
                            kT[h][:, k * KC:(k + 1) * KC], t[:])

        # ---- attention: flat stream of (qb, h, group), 3-deep pipeline ----
        pairs = [(qb, h) for qb in range(NQB) for h in range(2)]
        TOT = len(pairs) * NG

        with (
            tc.tile_pool(name="pt", bufs=8) as ptp,
            tc.tile_pool(name="on", bufs=8) as onp,
            tc.tile_pool(name="rs", bufs=2) as rsp,
            tc.tile_pool(name="osb", bufs=3) as osp,
            tc.tile_pool(name="sps", bufs=3, space="PSUM") as spp,
            tc.tile_pool(name="avps", bufs=1, space="PSUM") as avp,
        ):
            # PSUM allows only ONE accumulation group per 2KB bank (start=True
            # zeroes the whole bank).  Each pair gets a whole bank: the 4
            # q-subchunk AV accumulations form a single group (start on the
            # first (kc0,j0) matmul only, stop on the last (kc31,j3)), plus a
            # [40,128] transpose window at col 164 that is only written after
            # every normalize has read the av values (the transpose's
            # start=True wipes the bank).  Two banks alternate across pairs.
            avt2 = [avp.tile([128, 292], F32, tag=f"av{i}", name=f"av{i}")
                    for i in range(2)]
            p_tiles = {}

            def emit_scores_exp(Gi):
                qb, h = pairs[Gi // NG]
                g = Gi % NG
                sp = spp.tile([128, GRP * QB], F32, tag="sps", name="sps")
                q_sl = qT[h][:, qb * QB:(qb + 1) * QB]
                for i in range(GRP):
                    kc = g * GRP + i
                    nc.tensor.matmul(sp[:, i * QB:(i + 1) * QB],
                                     kT[h][:, kc * KC:(kc + 1) * KC], q_sl,
                                     start=True, stop=True)
                p = ptp.tile([128, GRP * QB], BF16, tag="pt", name="pt")
                if EXP_PAT[Gi % len(EXP_PAT)] == 0:
                    nc.scalar.activation(p[:], sp[:], EXPF, scale=float(SCALE))
                else:
                    nc.vector.tensor_scalar(
                        p[:].bitcast(I16), sp[:], float(A_EXP), float(B_EXP),
                        AluOpType.mult, AluOpType.add)
                p_tiles[Gi] = p

            def emit_av(Gi):
                pi = Gi // NG
                qb, h = pairs[pi]
                g = Gi % NG
                avt = avt2[pi % 2]
                p = p_tiles.pop(Gi)
                moff = 0 if h == 0 else DH + 1
                for i in range(GRP):
                    kc = g * GRP + i
                    for j in range(4):
                        o = j * (DH + 1)
                        nc.tensor.matmul(
                            avt[:, o:o + DH + 1],
                            p[:, i * QB + j * KC:i * QB + (j + 1) * KC],
                            vaug[:, kc, moff:moff + DH + 1],
                            start=(kc == 0 and j == 0),
                            stop=(kc == NKC - 1 and j == 3),
                            skip_group_check=True)

            tailq = []

            def enqueue_tail(pi):
                """Queue pair pi's normalize / transpose / out-projection as
                small closures, drained one per group iteration so the tail
                pipelines into the next pair's stream instead of stalling the
                in-order engines in a burst."""
                qb, h = pairs[pi]
                avt = avt2[pi % 2]
                doff = 0 if h == 0 else DH
                voff = 1 if h == 0 else 0
                r4 = rsp.tile([128, 4], F32, tag="r4", name="r4")
                ons = [onp.tile([128, DH], BF16, tag=f"on{j}", name=f"on{j}")
                       for j in range(4)]

                def recip(r4=r4, avt=avt, doff=doff):
                    with nc.allow_low_precision(reason="softmax denominator"):
                        nc.vector.reciprocal(
                            r4[:], avt[:, doff:doff + 4 * (DH + 1):DH + 1])
                tailq.append(recip)

                # every normalize (psum read) must precede any transpose:
                # the transpose's start=True zeroes the whole av bank
                for j in range(4):
                    def norm_j(r4=r4, on=ons[j], avt=avt, j=j, voff=voff):
                        o = j * (DH + 1) + voff
                        nc.scalar.mul(on[:], avt[:, o:o + DH], r4[:, j:j + 1])
                    tailq.append(norm_j)

                for j in range(4):
                    def tr_j(on=ons[j], avt=avt, h=h, st=qb * 4 + j, j=j):
                        t2 = avt[0:DH, 164:228].bitcast(BF16)
                        nc.tensor.transpose(t2, on[:], eye_t)
                        if j % 2 == 0:
                            nc.scalar.copy(
                                outT[h][:, st * KC:(st + 1) * KC], t2)
                        else:
                            nc.vector.tensor_copy(
                                outT[h][:, st * KC:(st + 1) * KC], t2)
                    tailq.append(tr_j)

                    if h == 1:
                        obc = []

                        def oprojA_j(obc=obc, st=qb * 4 + j):
                            op = spp.tile([128, GRP * QB], F32, tag="sps",
                                          name="op")
                            for hh in range(2):
                                nc.tensor.matmul(
                                    op[:, 0:DM],
                                    outT[hh][:, st * KC:(st + 1) * KC],
                                    wo_t[hh], start=(hh == 0),
                                    stop=(hh == 1))
                            ob = osp.tile([128, DM], BF16, tag="ob", name="ob")
                            nc.scalar.copy(ob[:], op[:, 0:DM])
                            obc.append(ob)
                        tailq.append(oprojA_j)

                        def oprojB_j(obc=obc, st=qb * 4 + j):
                            dq = nc.sync if st % 2 == 0 else nc.gpsimd
                            dq.dma_start(out_d[st * KC:(st + 1) * KC, :],
                                         obc[0][:])
                        tailq.append(oprojB_j)

            DEPTH = 3
            for Gi in range(TOT + DEPTH):
                if Gi < TOT:
                    emit_scores_exp(Gi)
                if Gi >= DEPTH:
                    emit_av(Gi - DEPTH)
                    if (Gi - DEPTH) % NG == NG - 1:
                        enqueue_tail((Gi - DEPTH) // NG)
                if tailq:
                    tailq.pop(0)()
            while tailq:
                tailq.pop(0)()

    nc.compile()
    return nc


def _get_nc():
    if "nc" not in _CACHE:
        _CACHE["nc"] = _build_nc()
    return _CACHE["nc"]


def _pack_wall(Wq, Wk, Wv, Wo, h0, bf16):
    wall = np.zeros((128, W_COLS), dtype=np.float32)
    for i in range(6):
        cs = slice(i * 128, (i + 1) * 128)
        wall[:, W_WKV + i * 160 + 0:W_WKV + i * 160 + 40] = \
            Wk[cs, h0 * DH:(h0 + 1) * DH]
        wall[:, W_WKV + i * 160 + 40:W_WKV + i * 160 + 80] = \
            Wk[cs, (h0 + 1) * DH:(h0 + 2) * DH]
        wall[:, W_WKV + i * 160 + 80:W_WKV + i * 160 + 120] = \
            Wv[cs, h0 * DH:(h0 + 1) * DH]
        wall[:, W_WKV + i * 160 + 120:W_WKV + i * 160 + 160] = \
            Wv[cs, (h0 + 1) * DH:(h0 + 2) * DH]
    for h in range(2):
        for i in range(3):
            o, n = [(0, 128), (128, 128), (256, 64)][i]
            wall[0:n, W_WQ + (3 * h + i) * 40:W_WQ + (3 * h + i + 1) * 40] = \
                Wq[o:o + n, (h0 + h) * DH:(h0 + h + 1) * DH]
        wall[0:DH, W_WO + h * 320:W_WO + (h + 1) * 320] = \
            Wo[(h0 + h) * DH:(h0 + h + 1) * DH, :]
    wall_bf = wall.astype(bf16)
    wall_bf[:, W_EYE:W_EYE + 128] = np.eye(128, dtype=bf16)
    return wall_bf


def kernel(x, cond, Wq, Wk, Wv, Wo, bo, _collect_results=None):
    import ml_dtypes

    bf16 = ml_dtypes.bfloat16
    x = np.asarray(x, dtype=np.float32)
    cond = np.asarray(cond, dtype=np.float32)
    Wq = np.asarray(Wq, dtype=np.float32)
    Wk = np.asarray(Wk, dtype=np.float32)
    Wv = np.asarray(Wv, dtype=np.float32)
    Wo = np.asarray(Wo, dtype=np.float32)
    bo = np.asarray(bo, dtype=np.float32)

    from concourse.bass_utils import run_bass_kernel_spmd

    nc = _get_nc()

    xt = np.zeros((2, 128, 3, S), dtype=bf16)
    ct = np.empty((2, 128, 6, SK), dtype=bf16)
    for b in range(2):
        xb = np.ascontiguousarray(x[b].T).astype(bf16)      # [320, S]
        xt[b, :, 0, :] = xb[0:128]
        xt[b, :, 1, :] = xb[128:256]
        xt[b, 0:64, 2, :] = xb[256:320]
        cb = np.ascontiguousarray(cond[b].T).astype(bf16)   # [768, SK]
        for i in range(6):
            ct[b, :, i, :] = cb[i * 128:(i + 1) * 128]

    in_maps = []
    for c in range(NCORES):
        b, h0 = c // 4, 2 * (c % 4)
        in_maps.append({
            "wall": _pack_wall(Wq, Wk, Wv, Wo, h0, bf16),
            "xT": xt[b],
            "condT": ct[b],
        })

    kw = _CACHE.pop("run_kwargs", {})
    res = run_bass_kernel_spmd(nc, in_maps, core_ids=list(range(NCORES)), **kw)
    if _collect_results is not None:
        _collect_results.append(res)
    outs = [r["out"].astype(np.float32) for r in res.results]
    full = np.stack([
        outs[0] + outs[1] + outs[2] + outs[3],
        outs[4] + outs[5] + outs[6] + outs[7],
    ])
    return full + bo[None, None, :]
